# revision 1
# baseline (speedup 1.0000x reference)
"""Trainium2 Bass kernel: spiking multi-head attention (nn_MultiHeadedAttention).

Reference semantics (B=4, T=2048, DIN=100, D=512, h=8 heads, dk=64):
    q = spike(query @ Wq + bq)   (spike = (x >= 1.0) -> {0,1})
    k = spike(key @ Wk + bk);  v = spike(value @ Wv + bv)
    attn = (q @ k^T) * scale, causally masked (keep k<=q), NO softmax
    x = spike(attn @ v)
    x = x.transpose(0,1,3,2).reshape(B,T,h*dk)    # scrambled reshape
    y = spike(x @ Wo + bo)

Key facts exploited:
  * No softmax -> causal attention is LINEAR attention:
        O_t = q_t . M_t  +  intra-block tril(Q K^T) V,   M = sum_j k_j v_j^T
    The running 64x64/head state M accumulates in PSUM across 16 t-blocks,
    so only 16 diagonal 128x128 S-tiles per head are ever materialized.
  * The scrambled reshape maps output rows [256*h, 256*(h+1)) to exactly one
    head h, so head-parallel sharding needs NO cross-core communication.
  * Spiked tensors are {0,1} and S is integer <=64, so bf16 matmul operands
    with fp32 PSUM accumulation are bit-exact.  fp32 matmuls cost 2 PE
    passes, so every exactly-representable operand pair runs in bf16; the
    t-major spiked K comes from DMA-xbar transposes of the bf16 d-major K
    instead of a second fp32 projection.

Sharding: core c -> batch b=c//2, head-group hg=c%2 (4 heads per core).

Hardware pitfalls encoded below:
  * K=64 matmuls whose lhsT sits at partition base 0 vs base 64 execute
    concurrently in disjoint PE row groups; concurrent writes to one PSUM
    bank hang the device.  Even-head (base 0) and odd-head (base 64) K=64
    outputs therefore always target different banks; K=128 matmuls between
    them act as barriers (full row occupancy).
  * start=True zeroes a whole 2KB PSUM bank region, so co-located
    accumulation groups share a single start.
  * DMA-issue instructions cost ~0.6us each on the issuing engine, so all
    weights ride one packed transfer and k/v loads issue from GPSIMD.
"""

import os
import numpy as np

B, T, DIN, D = 4, 2048, 100, 512
H, DK = 8, 64
NCORES = 8
HPC = 4          # heads per core
DH = HPC * DK    # 256 projected features per core
P = 128
NT = T // P      # 16 t-blocks
KC = D // P      # 4 contraction chunks of the D=512 dim
NPIECE = 4       # DMA load pieces along T

# packed-weights column offsets (fp32 columns of the [128, WPACK_W] tensor)
OFF_WK = 0
OFF_WV = 1024
OFF_WQ = 2048
OFF_MSK = 2304
OFF_WO = 2560
OFF_BIAS = 4608
WPACK_W = 5120

_prog_cache: dict = {}
last_exec_time_ns = None


def _build(scale: float, has_bk: bool, has_bv: bool, has_bo: bool):
    from contextlib import ExitStack

    import concourse.bass as bass
    import concourse.tile as tile
    import concourse.mybir as mybir
    from concourse import bacc
    from concourse.bass import ts
    from concourse import masks

    f32 = mybir.dt.float32
    f16 = mybir.dt.float16
    ALU = mybir.AluOpType
    AF = mybir.ActivationFunctionType
    BIG = float(2 ** 26)

    nc = bacc.Bacc(
        "TRN2", target_bir_lowering=False, debug=False, num_devices=NCORES
    )

    # DRAM I/O (host pre-transposes; qT/wq carry an extra bias row; all
    # weights/mask/bias packed into one tensor = one DMA issue).
    qT = nc.dram_tensor("qT", [DIN + 1, T], f32, kind="ExternalInput").ap()
    kT = nc.dram_tensor("kT", [D, T], f32, kind="ExternalInput").ap()
    vT = nc.dram_tensor("vT", [D, T], f32, kind="ExternalInput").ap()
    wpk = nc.dram_tensor("wpk", [P, WPACK_W], f32, kind="ExternalInput").ap()
    y = nc.dram_tensor("y", [HPC * 256, D], f32, kind="ExternalOutput").ap()

    with tile.TileContext(nc) as tc, ExitStack() as ctx:
        pool = lambda name, bufs, space="SBUF": ctx.enter_context(
            tc.tile_pool(name=name, bufs=bufs, space=space)
        )
        persist = pool("persist", 1)      # distinct tags -> own slots
        s_pool = pool("s_pool", 4)        # masked S tiles (bf16)
        t_pool = pool("t_pool", 4)        # ACT-chain temporaries
        m_pool = pool("m_pool", 2)        # M snapshots
        y_pool = pool("y_pool", 3)        # output staging
        pp = pool("pp", 3, "PSUM")        # projections/final/transposes
        ps = pool("ps", 1, "PSUM")        # S^T tiles (2 parity tags)
        po = pool("po", 2, "PSUM")        # O accumulators
        pm = pool("pm", 1, "PSUM")        # persistent M state

        def ptile(shape, dtype=f32, *, name):
            return persist.tile(shape, dtype, name=name, tag=name)

        # ---- SBUF allocations -----------------------------------------
        qt_sb = ptile([P, T], name="qt_sb")
        kt_sb = [ptile([P, T], name=f"kt_sb{c}") for c in range(KC)]
        vt_sb = [ptile([P, T], name=f"vt_sb{c}") for c in range(KC)]
        wp_sb = ptile([P, WPACK_W], name="wp_sb")
        wk_sb = [wp_sb[:, OFF_WK + 256 * c :][:, 0:DH] for c in range(KC)]
        wv_sb = [wp_sb[:, OFF_WV + 256 * c :][:, 0:DH] for c in range(KC)]
        wq_sb = wp_sb[:, OFF_WQ : OFF_WQ + DH]
        msk_sb = wp_sb[:, OFF_MSK : OFF_MSK + DH]
        wo_sb = [wp_sb[:, OFF_WO + 512 * c :][:, 0:D] for c in range(KC)]
        bias_sb = wp_sb[:, OFF_BIAS : OFF_BIAS + D]
        ones_sb = ptile([1, D], name="ones_sb")
        idt_sb = ptile([P, P], f16, name="idt_sb")
        # qs/ks: spiked projections, d-major [dk, T]; tile i holds heads
        # 2i (parts 0:64) and 2i+1 (parts 64:128).  fp16: {0,1} and the
        # integer M state (<= 2048 < 2^11) are exact, and fp16 matmuls run
        # 1 cycle/row vs fp32's 4.
        qs = [ptile([P, T], f16, name=f"qs{i}") for i in range(2)]
        ks = [ptile([P, T], f16, name=f"ks{i}") for i in range(2)]
        # vkn: t-major spiked v for all 4 heads (cols 256t+64*hl), bf16.
        vkn = ptile([P, DH * NT], f16, name="vkn")
        # kn: t-major spiked k via DMA-xbar transpose of ks, pair-major:
        # cols 256t + 128*pair + 64*(hl%2)
        kn = ptile([P, DH * NT], f16, name="kn")
        # xs: spiked attention output, laid out xs[p, 1024h + 16d + t_blk]
        # so the final-projection lhsT view has a single stride-4 free dim.
        xs = ptile([P, 1024 * HPC], name="xs")

        # ---- loads ----------------------------------------------------
        # The DMA ring fair-shares bandwidth across all in-flight
        # transfers, so later loads are throttled behind earlier ones with
        # tiny gate-copies (read prev dest, write next dest: RAW + WAR).
        nc.sync.dma_start(out=wp_sb[:, 0:OFF_WO], in_=wpk[:, 0:OFF_WO])
        for qc in range(4):
            nc.sync.dma_start(
                out=qt_sb[: DIN + 1, ts(qc, 512)], in_=qT[:, ts(qc, 512)]
            )
        nc.vector.memset(ones_sb[:, :], 1.0)
        masks.make_identity(nc, idt_sb[:, :])
        PW = T // NPIECE
        for pc in range(NPIECE):
            for c in range(KC):
                nc.sync.dma_start(
                    out=kt_sb[c][:, ts(pc, PW)], in_=kT[ts(c, P), ts(pc, PW)]
                )
            for c in range(KC):
                nc.sync.dma_start(
                    out=vt_sb[c][:, ts(pc, PW)], in_=vT[ts(c, P), ts(pc, PW)]
                )
        nc.gpsimd.tensor_copy(
            wp_sb[0:1, OFF_WO : OFF_WO + 1], vt_sb[KC - 1][0:1, T - PW : T - PW + 1]
        )
        nc.sync.dma_start(
            out=wp_sb[:, OFF_WO:WPACK_W], in_=wpk[:, OFF_WO:WPACK_W]
        )

        def spike_act(out_ap, in_ap, nm):
            """out = (in >= 1.0) via two exact Relu ops on the ACT engine."""
            tmp = t_pool.tile(list(out_ap.shape), f32, name=f"tmp_{nm}")
            nc.scalar.activation(tmp[:, :], in_ap, AF.Relu, bias=1.0, scale=-1.0)
            nc.scalar.activation(out_ap, tmp[:, :], AF.Relu, bias=1.0, scale=-BIG)

        # ---- qs projection (only needs qt) ----------------------------
        for half in range(2):
            for ch in range(KC):
                pt = pp.tile([P, 512], f32, name="pt", tag="pt")
                nc.tensor.matmul(
                    pt[:, :],
                    lhsT=wq_sb[: DIN + 1, ts(half, P)],
                    rhs=qt_sb[: DIN + 1, ts(ch, 512)],
                    start=True,
                    stop=True,
                )
                spike_act(qs[half][:, ts(ch, 512)], pt[:, :], "q")

        # ---- pipelined: per piece, ks chunk -> vkn blocks -> attention -
        pm_t = pm.tile([P, DH], f32, name="pm_t")
        xs_r = xs.rearrange(
            "p (he par d t) -> p par he d t", he=2, par=2, d=DK, t=NT
        )

        def ks_chunk(ch):
            for half in range(2):
                pt = pp.tile([P, 512], f32, name="pt", tag="pt")
                for c in range(KC):
                    nc.tensor.matmul(
                        pt[:, :],
                        lhsT=wk_sb[c][:, ts(half, P)],
                        rhs=kt_sb[c][:, ts(ch, 512)],
                        start=(c == 0),
                        stop=(c == KC - 1) and not has_bk,
                    )
                if has_bk:
                    nc.tensor.matmul(
                        pt[:, :],
                        lhsT=bias_sb[0:1, ts(half, P)],
                        rhs=ones_sb[0:1, 0:512],
                        start=False,
                        stop=True,
                    )
                spike_act(ks[half][:, ts(ch, 512)], pt[:, :], "k")
            # t-major spiked K for this chunk's 4 blocks via PE transpose
            # (fp16, 1 cycle/row); a [128,128] head-pair tile transpose
            # lands exactly in the pair-major layout the M-update wants.
            for tt in range(4 * ch, 4 * ch + 4):
                for pr in range(2):
                    tp = pp.tile([P, P], f16, name="tp", tag="pt")
                    nc.tensor.transpose(
                        tp[:, :], ks[pr][:, ts(tt, P)], idt_sb[:, :]
                    )
                    nc.vector.tensor_copy(
                        kn[:, DH * tt + P * pr :][:, 0:P], tp[:, :]
                    )

        def vkn_block(tt):
            pt = pp.tile([P, 512], f32, name="pt", tag="pt")
            for c in range(KC):
                nc.tensor.matmul(
                    pt[:, 0:DH],
                    lhsT=vt_sb[c][:, ts(tt, P)],
                    rhs=wv_sb[c][:, :],
                    start=(c == 0),
                    stop=(c == KC - 1) and not has_bv,
                )
            if has_bv:
                nc.tensor.matmul(
                    pt[:, 0:DH],
                    lhsT=ones_sb[0:1, 0:P],
                    rhs=bias_sb[1:2, 0:DH],
                    start=False,
                    stop=True,
                )
            nc.vector.tensor_scalar(
                vkn[:, ts(tt, DH)], pt[:, 0:DH], 1.0, None, ALU.is_ge
            )

        def attn_block(tt):
            if tt > 0:
                # snapshot M_(<tt); single [128,256] copy covers both
                # partition halves (diagonal 64x64 blocks hold real M)
                m_sb = m_pool.tile([P, DH], f16, name="m_sb")
                nc.scalar.copy(m_sb[:, :], pm_t[:, :])
            else:
                m_sb = None
            s_ps = [
                ps.tile([P, DH], f32, name=f"s_ps{par}", tag=f"s_ps{par}")
                for par in range(2)
            ]
            for hl in range(HPC):
                par, idx = hl % 2, hl // 2
                rows = slice(64 * par, 64 * par + 64)
                nc.tensor.matmul(
                    s_ps[par][:, ts(idx, P)],
                    lhsT=ks[idx][rows, ts(tt, P)],
                    rhs=qs[idx][rows, ts(tt, P)],
                    start=True,
                    stop=True,
                )
            s_sb = [
                s_pool.tile([P, DH], f16, name=f"s_sb{par}", tag=f"s_sb{par}")
                for par in range(2)
            ]
            for par in range(2):
                nc.vector.tensor_tensor(
                    s_sb[par][:, :], s_ps[par][:, :], msk_sb[:, :], op=ALU.mult
                )
            o_ps = [po.tile([P, P], f32, name="o_ps") for _ in range(2)]
            for hl in range(HPC):
                par, idx = hl % 2, hl // 2
                rows = slice(64 * par, 64 * par + 64)
                nc.tensor.matmul(
                    o_ps[par][:, ts(idx, 64)],
                    lhsT=s_sb[par][:, ts(idx, P)],
                    rhs=vkn[:, DH * tt + 64 * hl :][:, 0:64],
                    start=True,
                    stop=(tt == 0),
                )
                if tt > 0:
                    mc = 128 * idx + 64 * par
                    nc.tensor.matmul(
                        o_ps[par][:, ts(idx, 64)],
                        lhsT=qs[idx][rows, ts(tt, P)],
                        rhs=m_sb[rows, mc : mc + 64],
                        start=False,
                        stop=True,
                    )
            # M += K_pair^T V_pair: one K=128,N=128 matmul per head pair;
            # only the diagonal 64x64 blocks are meaningful.  stop=True
            # each block closes the sim's accumulation group so the
            # snapshot read is legal; on HW stop is a no-op and the
            # start=False matmuls keep accumulating.
            for pr in range(2):
                nc.tensor.matmul(
                    pm_t[:, ts(pr, P)],
                    lhsT=kn[:, DH * tt + P * pr :][:, 0:P],
                    rhs=vkn[:, DH * tt + P * pr :][:, 0:P],
                    start=(tt == 0 and pr == 0),
                    stop=(pr == 1),
                    skip_group_check=True,
                )
            # x = spike(scale * O) = (relu(1 - scale*O) <= 0): exact, and
            # splits across the idle ACT/GPSIMD engines.
            for par in range(2):
                xtmp = t_pool.tile([P, P], f32, name="xtmp")
                nc.scalar.activation(
                    xtmp[:, :], o_ps[par][:, :], AF.Relu,
                    bias=1.0, scale=-float(scale),
                )
                nc.vector.tensor_scalar(
                    xs_r[:, par, :, :, tt],
                    xtmp[:, :].rearrange("p (h d) -> p h d", h=2),
                    0.0,
                    None,
                    ALU.is_le,
                )

        def proj_piece(pc):
            ks_chunk(pc)
            for tt in range(4 * pc, 4 * pc + 4):
                vkn_block(tt)

        proj_piece(0)
        proj_piece(1)
        # Final projection runs per piece: output rows r with r%4 == m
        # contract only over attention piece m (X[r, f] =
        # x_att[t=512*(r%4)+f, d=r//4]).  xs col = 16*(64h + r//4) +
        # (4m + cc), so a head PAIR's 128 rows are one stride-16 lhsT.
        xq = xs.rearrange("p (q mc) -> p mc q", q=256, mc=16)

        def final_piece(m):
            for j in range(2):  # head pair: heads 2j, 2j+1
                yp = pp.tile([P, 512], f32, name="pt", tag="pt")
                for cc in range(KC):
                    nc.tensor.matmul(
                        yp[:, :],
                        lhsT=xq[:, 4 * m + cc, ts(j, P)],
                        rhs=wo_sb[cc][:, :],
                        start=(cc == 0),
                        stop=(cc == KC - 1) and not has_bo,
                    )
                if has_bo:
                    nc.tensor.matmul(
                        yp[:, :],
                        lhsT=ones_sb[0:1, 0:P],
                        rhs=bias_sb[2:3, :],
                        start=False,
                        stop=True,
                    )
                y_sb = y_pool.tile([P, D], f32, name="y_sb")
                nc.vector.tensor_scalar(
                    y_sb[:, :], yp[:, :], 1.0, None, ALU.is_ge
                )
                for sub in range(2):
                    h = 2 * j + sub
                    nc.gpsimd.dma_start(
                        out=y[256 * h + m : 256 * (h + 1) : 4, :],
                        in_=y_sb[64 * sub : 64 * sub + 64, :],
                    )

        for pc in range(4):
            if pc + 2 < 4:
                proj_piece(pc + 2)
            for tt in range(4 * pc, 4 * pc + 4):
                attn_block(tt)
            final_piece(pc)

    nc.compile()
    return nc


def _get_prog(scale, has_bk, has_bv, has_bo):
    key = (scale, has_bk, has_bv, has_bo)
    if key not in _prog_cache:
        _prog_cache[key] = _build(scale, has_bk, has_bv, has_bo)
    return _prog_cache[key]


def _pack_weights(Wq, bq, Wk, bk, Wv, bv, Wo, bo, cs):
    wpk = np.zeros((P, WPACK_W), np.float32)
    for c in range(KC):
        wpk[:, OFF_WK + 256 * c : OFF_WK + 256 * (c + 1)] = Wk[
            128 * c : 128 * (c + 1), cs
        ]
        wpk[:, OFF_WV + 256 * c : OFF_WV + 256 * (c + 1)] = Wv[
            128 * c : 128 * (c + 1), cs
        ]
        wpk[:, OFF_WO + 512 * c : OFF_WO + 512 * (c + 1)] = Wo[
            128 * c : 128 * (c + 1), :
        ]
    wpk[:DIN, OFF_WQ : OFF_WQ + DH] = Wq[:, cs]
    wpk[DIN, OFF_WQ : OFF_WQ + DH] = bq[cs]
    wpk[:, OFF_MSK : OFF_MSK + DH] = np.tile(
        np.triu(np.ones((P, P), np.float32)), (1, 2)
    )
    wpk[0, OFF_BIAS : OFF_BIAS + DH] = bk[cs]
    wpk[1, OFF_BIAS : OFF_BIAS + DH] = bv[cs]
    wpk[2, OFF_BIAS : OFF_BIAS + D] = bo
    return wpk


def kernel(**inputs) -> np.ndarray:
    global last_exec_time_ns
    from concourse.bass_utils import run_bass_kernel_spmd

    g = lambda n: np.asarray(inputs[n], dtype=np.float32)
    query, key, value = g("query"), g("key"), g("value")
    Wq, bq, Wk, bk = g("Wq"), g("bq"), g("Wk"), g("bk")
    Wv, bv, Wo, bo = g("Wv"), g("bv"), g("Wo"), g("bo")
    scale = float(np.asarray(inputs["scale"], dtype=np.float32).reshape(-1)[0])

    has_bk, has_bv, has_bo = (bool(np.any(x)) for x in (bk, bv, bo))
    prog = _get_prog(scale, has_bk, has_bv, has_bo)

    in_maps = []
    for c in range(NCORES):
        b, hg = divmod(c, 2)
        cs = slice(DH * hg, DH * (hg + 1))
        qTa = np.empty((DIN + 1, T), np.float32)
        qTa[:DIN] = query[b].T
        qTa[DIN] = 1.0
        in_maps.append(
            {
                "qT": qTa,
                "kT": np.ascontiguousarray(key[b].T),
                "vT": np.ascontiguousarray(value[b].T),
                "wpk": _pack_weights(Wq, bq, Wk, bk, Wv, bv, Wo, bo, cs),
            }
        )

    trace = os.environ.get("BASS_TRACE", "") not in ("", "0")
    res = run_bass_kernel_spmd(
        prog, in_maps, core_ids=list(range(NCORES)), trace=trace
    )
    last_exec_time_ns = res.exec_time_ns
    if res.exec_time_ns is not None:
        print(f"HW exec time: {res.exec_time_ns} ns")

    out = np.empty((B, T, D), np.float32)
    for c in range(NCORES):
        b, hg = divmod(c, 2)
        out[b, 1024 * hg : 1024 * (hg + 1)] = res.results[c]["y"]
    return out

